# revision 1
# baseline (speedup 1.0000x reference)
import sys

sys.path.insert(0, "/opt/trn_rl_repo")

import numpy as np

# Problem constants (hardcoded per contract)
B, L, C, K = 8, 16384, 64, 7
T = (L - 2 * K) // 2 + 1  # 8186
HALF = 4096               # t's per half (half-1 ragged: 8186-4096=4090, padded)
TC = 512                  # t-chunk
NCH = HALF // TC          # 8 chunks
WX = 4104                 # column width of folded x tensors (HALF + 8 pad)
LN_EPS = 1e-6

_CACHE = {}


def _build(prelu_slope: float, need_lnsb: bool, need_cb: bool):
    import concourse.bacc as bacc
    import concourse.mybir as mybir
    import concourse.tile as tile

    f32 = mybir.dt.float32
    f16 = mybir.dt.float16
    AF = mybir.ActivationFunctionType
    OP = mybir.AluOpType

    nc = bacc.Bacc("TRN2", target_bir_lowering=False, debug=False, num_devices=8)

    # ---- DRAM parameters (per-core shard data) ----
    dXE = nc.declare_dram_parameter("xe", [128, WX], f16, isOutput=False)
    dXE1 = nc.declare_dram_parameter("xe1", [128, WX], f16, isOutput=False)
    dXO = nc.declare_dram_parameter("xo", [128, WX], f16, isOutput=False)
    dXO1 = nc.declare_dram_parameter("xo1", [128, WX], f16, isOutput=False)
    dWT = nc.declare_dram_parameter("wt", [128, 64 * K], f16, isOutput=False)
    dID = nc.declare_dram_parameter("ident", [128, 128], f16, isOutput=False)
    dON = nc.declare_dram_parameter("ones64", [128, 64], f16, isOutput=False)
    dCK = nc.declare_dram_parameter("ck", [128, 64], f16, isOutput=False)
    dCST = nc.declare_dram_parameter("csts", [128, 4], f32, isOutput=False)
    dOUT = nc.declare_dram_parameter("out", [T, C], f32, isOutput=True)

    from contextlib import ExitStack

    with ExitStack() as es:
        tc = es.enter_context(tile.TileContext(nc))
        cp = es.enter_context(tc.tile_pool(name="const", bufs=1))
        gp = es.enter_context(tc.tile_pool(name="gps", bufs=2, space="PSUM"))
        yp = es.enter_context(tc.tile_pool(name="yps", bufs=1, space="PSUM"))
        zp = es.enter_context(tc.tile_pool(name="zps", bufs=1, space="PSUM"))
        sp = es.enter_context(tc.tile_pool(name="sps", bufs=1, space="PSUM"))
        hp = es.enter_context(tc.tile_pool(name="hsb", bufs=10))
        pp = es.enter_context(tc.tile_pool(name="prod", bufs=16))
        ypool = es.enter_context(tc.tile_pool(name="ysb", bufs=3))
        st1 = es.enter_context(tc.tile_pool(name="st1", bufs=3))
        st2 = es.enter_context(tc.tile_pool(name="st2", bufs=3))
        st3 = es.enter_context(tc.tile_pool(name="st3", bufs=3))
        st4 = es.enter_context(tc.tile_pool(name="st4", bufs=3))
        st5 = es.enter_context(tc.tile_pool(name="st5", bufs=3))
        ynp = es.enter_context(tc.tile_pool(name="ynp", bufs=3))
        pzp = es.enter_context(tc.tile_pool(name="pzp", bufs=3))
        trp = es.enter_context(tc.tile_pool(name="trp", bufs=6))
        op_ = es.enter_context(tc.tile_pool(name="outp", bufs=4))
        if True:
            # ---- load constants ----
            XE = cp.tile([128, WX], f16)
            XE1 = cp.tile([128, WX], f16)
            XO = cp.tile([128, WX], f16)
            XO1 = cp.tile([128, WX], f16)
            WT = cp.tile([128, 64 * K], f16)
            ID = cp.tile([128, 128], f16)
            ON = cp.tile([128, 64], f16)
            CKt = cp.tile([128, 64], f16)
            CST = cp.tile([128, 4], f32)
            for t_, d_ in ((XE, dXE), (XE1, dXE1), (XO, dXO), (XO1, dXO1),
                           (WT, dWT), (ID, dID), (ON, dON), (CKt, dCK), (CST, dCST)):
                nc.sync.dma_start(t_[:], d_[:])

            for i in range(NCH):
                t0 = TC * i
                # ---- G matmuls + tanh: 7 m-planes, each (Ge|Go) (128,1024) ----
                hts = []
                for m in range(K):
                    g = gp.tile([128, 1024], f32)
                    for ci, src_ in ((0, XE), (512, XO)):
                        for h in (0, 1):
                            p0 = 64 * h
                            nc.tensor.matmul(
                                g[p0:p0 + 64, ci:ci + TC],
                                lhsT=WT[p0:p0 + 64, 64 * m:64 * m + 64],
                                rhs=src_[p0:p0 + 64, t0 + 6:t0 + 6 + TC],
                                start=True, stop=True,
                            )
                    ht = hp.tile([128, 1024], f16)
                    nc.scalar.activation(ht[:], g[:], AF.Tanh)
                    hts.append(ht)

                # ---- gating products (14 planes) ----
                prods = []
                for m in range(K):
                    for ci, (xa, xs) in ((0, (XE, XE1)), (512, (XO, XO1))):
                        pr = pp.tile([128, TC], f16)
                        if m % 2 == 0:
                            xap = xa[:, t0 + m:t0 + m + TC]
                        else:
                            xap = xs[:, t0 + m - 1:t0 + m - 1 + TC]
                        nc.vector.tensor_mul(pr[:], xap, hts[m][:, ci:ci + TC])
                        prods.append(pr)

                # ---- accumulate 14 products + skip via identity matmuls ----
                y = yp.tile([128, TC], f32)
                for j, pr in enumerate(prods):
                    nc.tensor.matmul(y[:], lhsT=ID[:], rhs=pr[:],
                                     start=(j == 0), stop=False)
                nc.tensor.matmul(y[:], lhsT=ID[:],
                                 rhs=XE[:, t0 + 6:t0 + 6 + TC],
                                 start=False, stop=True)

                # ---- drain y, square ----
                ysb = ypool.tile([128, TC], f16)
                nc.scalar.copy(ysb[:], y[:])
                ysq = pp.tile([128, TC], f16)
                nc.vector.tensor_mul(ysq[:], ysb[:], ysb[:])

                # ---- LN stats: mean & mean-of-squares via ones-matmul ----
                st = sp.tile([128, 1024], f32)
                for h in (0, 1):
                    p0 = 64 * h
                    nc.tensor.matmul(st[p0:p0 + 64, 0:TC],
                                     lhsT=ON[p0:p0 + 64, :],
                                     rhs=ysb[p0:p0 + 64, :], start=True, stop=True)
                    nc.tensor.matmul(st[p0:p0 + 64, 512:512 + TC],
                                     lhsT=ON[p0:p0 + 64, :],
                                     rhs=ysq[p0:p0 + 64, :], start=True, stop=True)
                mu = st[:, 0:TC]
                m2 = st[:, 512:512 + TC]

                musq = st1.tile([128, TC], f32)
                nc.scalar.activation(musq[:], mu, AF.Square)
                var = st2.tile([128, TC], f32)
                nc.vector.tensor_sub(var[:], m2, musq[:])
                std = st3.tile([128, TC], f32)
                nc.scalar.activation(std[:], var[:], AF.Sqrt, bias=CST[:, 3:4])
                rstd = st4.tile([128, TC], f32)
                scr = st5.tile([128, TC], f32)
                nc.vector.reciprocal_approx_accurate(rstd[:], std[:], scr[:])

                # ---- yn = (y - mu) * rstd  (* s + b) ----
                yc = st1.tile([128, TC], f32)
                nc.vector.tensor_sub(yc[:], ysb[:], mu)
                yn = ynp.tile([128, TC], f16)
                nc.vector.tensor_mul(yn[:], yc[:], rstd[:])
                if need_lnsb:
                    yn2 = ynp.tile([128, TC], f16)
                    nc.vector.tensor_scalar(yn2[:], yn[:], CST[:, 0:1], CST[:, 1:2],
                                            op0=OP.mult, op1=OP.add)
                    yn = yn2

                # ---- 1x1 conv ----
                z = zp.tile([128, TC], f32)
                for h in (0, 1):
                    p0 = 64 * h
                    nc.tensor.matmul(z[p0:p0 + 64, :], lhsT=CKt[p0:p0 + 64, :],
                                     rhs=yn[p0:p0 + 64, :], start=True, stop=True)
                if need_cb:
                    z2 = st2.tile([128, TC], f32)
                    nc.vector.tensor_scalar(z2[:], z[:], CST[:, 2:3], None, op0=OP.add)
                    zsrc = z2
                else:
                    zsrc = z
                # prelu: max(z, slope*z)
                pz = pzp.tile([128, TC], f16)
                nc.scalar.activation(pz[:], zsrc[:], AF.Prelu,
                                     alpha=float(prelu_slope))

                # ---- transpose yn, pz to t-layout; add; store ----
                for h in (0, 1):
                    p0 = 64 * h
                    tb = HALF * h + t0
                    ynT = trp.tile([128, 4, 64], f16)
                    nc.sync.dma_start_transpose(ynT[:], yn[p0:p0 + 64, :])
                    pzT = trp.tile([128, 4, 64], f16)
                    nc.sync.dma_start_transpose(pzT[:], pz[p0:p0 + 64, :])
                    of = op_.tile([128, 4, 64], f32)
                    nc.vector.tensor_add(of[:], ynT[:], pzT[:])
                    if tb + TC <= T:
                        dst = dOUT[tb:tb + TC, :].rearrange(
                            "(j p) c -> p j c", p=128)
                        nc.sync.dma_start(dst, of[:])
                    else:
                        nfull = (T - tb) // 128
                        rem = (T - tb) - nfull * 128
                        if nfull > 0:
                            dst = dOUT[tb:tb + nfull * 128, :].rearrange(
                                "(j p) c -> p j c", p=128)
                            nc.sync.dma_start(dst, of[:, 0:nfull, :])
                        if rem > 0:
                            dst = dOUT[tb + nfull * 128:T, :]
                            nc.sync.dma_start(dst, of[0:rem, nfull, :])

    nc.compile()
    return nc


def _prep_inputs(x, weights, ln_scale, ln_bias, conv_kernel, conv_bias):
    """Host-side prep: returns (per-core input maps, shared consts)."""
    xf = np.asarray(x, dtype=np.float32)
    # shared consts
    WT = np.zeros((128, 64 * K), np.float16)
    for m in range(K):
        wmT = np.asarray(weights[:, :, m]).T.astype(np.float16)  # (c_in, d)
        WT[0:64, 64 * m:64 * m + 64] = wmT
        WT[64:128, 64 * m:64 * m + 64] = wmT
    ID = np.eye(128, dtype=np.float16)
    ON = np.full((128, 64), 1.0 / 64, np.float16)
    CK = np.zeros((128, 64), np.float16)
    ckc = np.asarray(conv_kernel).astype(np.float16)  # (c, o), lhsT layout
    CK[0:64] = ckc
    CK[64:128] = ckc
    CST = np.zeros((128, 4), np.float32)
    s = np.asarray(ln_scale, np.float32)
    b = np.asarray(ln_bias, np.float32)
    cb = np.asarray(conv_bias, np.float32)
    CST[0:64, 0] = s
    CST[64:128, 0] = s
    CST[0:64, 1] = b
    CST[64:128, 1] = b
    CST[0:64, 2] = cb
    CST[64:128, 2] = cb
    CST[:, 3] = LN_EPS

    def fold(a):  # a: (64, 8192) -> (128, WX)
        out = np.zeros((128, WX), np.float16)
        out[0:64, :] = a[:, 0:WX]
        out[64:128, 0:8192 - HALF] = a[:, HALF:8192]
        return out

    in_maps = []
    for bi in range(B):
        xb = xf[bi]                      # (L, C)
        xeT = np.ascontiguousarray(xb[0::2].T).astype(np.float16)  # (64, 8192)
        xoT = np.ascontiguousarray(xb[1::2].T).astype(np.float16)
        xeT1 = np.concatenate([xeT[:, 1:], np.zeros((64, 1), np.float16)], axis=1)
        xoT1 = np.concatenate([xoT[:, 1:], np.zeros((64, 1), np.float16)], axis=1)
        in_maps.append({
            "xe": fold(xeT), "xe1": fold(xeT1),
            "xo": fold(xoT), "xo1": fold(xoT1),
            "wt": WT, "ident": ID, "ones64": ON, "ck": CK, "csts": CST,
        })
    return in_maps


def kernel(x, weights, ln_scale, ln_bias, conv_kernel, conv_bias, prelu_slope):
    from concourse.bass_utils import run_bass_kernel_spmd

    slope = float(np.asarray(prelu_slope))
    need_lnsb = not (np.allclose(np.asarray(ln_scale), 1.0)
                     and np.allclose(np.asarray(ln_bias), 0.0))
    need_cb = not np.allclose(np.asarray(conv_bias), 0.0)

    key = (slope, need_lnsb, need_cb)
    if key not in _CACHE:
        _CACHE[key] = _build(slope, need_lnsb, need_cb)
    nc = _CACHE[key]

    in_maps = _prep_inputs(x, weights, ln_scale, ln_bias, conv_kernel, conv_bias)
    res = run_bass_kernel_spmd(nc, in_maps, core_ids=list(range(8)))
    out = np.stack([res.results[i]["out"] for i in range(B)], axis=0)
    return out.astype(np.float32)



# revision 5
# speedup vs baseline: 10.0096x; 10.0096x over previous
import sys

sys.path.insert(0, "/opt/trn_rl_repo")

import numpy as np

# Problem constants (hardcoded per contract)
B, L, C, K = 8, 16384, 64, 7
T = (L - 2 * K) // 2 + 1  # 8186
HALF = 4096               # t's per half (half-1 ragged: 8186-4096=4090, padded)
TC = 512                  # t-chunk
NCH = HALF // TC          # 8 chunks
WX = 4104                 # column width of folded x tensors (HALF + 8 pad)
LN_EPS = 1e-6

# single-blob input column offsets (all f16)
O_XE = 0
O_XO = WX                 # 4104
O_WT = 2 * WX             # 8208
O_ID = O_WT + 64 * K      # 8656
O_ON = O_ID + 128         # 8784
O_CK = O_ON + 64          # 8848
NB = O_CK + 64            # 8912

_CACHE = {}


def _build(prelu_slope: float, need_lnsb: bool, need_cb: bool):
    import concourse.bacc as bacc
    import concourse.mybir as mybir
    import concourse.tile as tile

    f32 = mybir.dt.float32
    f16 = mybir.dt.float16
    AF = mybir.ActivationFunctionType
    OP = mybir.AluOpType

    nc = bacc.Bacc("TRN2", target_bir_lowering=False, debug=False, num_devices=8)

    # ---- DRAM parameters ----
    dBLOB = nc.declare_dram_parameter("blob", [128, NB], f16, isOutput=False)
    if need_lnsb or need_cb:
        dCST = nc.declare_dram_parameter("csts", [128, 4], f32, isOutput=False)
    dOUT = nc.declare_dram_parameter("out", [T, C], f16, isOutput=True)

    from contextlib import ExitStack

    with ExitStack() as es:
        tc = es.enter_context(tile.TileContext(nc))
        cp = es.enter_context(tc.tile_pool(name="const", bufs=1))
        gp = es.enter_context(tc.tile_pool(name="gps", bufs=2, space="PSUM"))
        yp = es.enter_context(tc.tile_pool(name="yps", bufs=1, space="PSUM"))
        zp = es.enter_context(tc.tile_pool(name="zps", bufs=1, space="PSUM"))
        sp = es.enter_context(tc.tile_pool(name="sps", bufs=1, space="PSUM"))
        hp = es.enter_context(tc.tile_pool(name="hsb", bufs=10))
        pp = es.enter_context(tc.tile_pool(name="prod", bufs=16))
        ypool = es.enter_context(tc.tile_pool(name="ysb", bufs=3))
        st1 = es.enter_context(tc.tile_pool(name="st1", bufs=3))
        st2 = es.enter_context(tc.tile_pool(name="st2", bufs=3))
        st3 = es.enter_context(tc.tile_pool(name="st3", bufs=3))
        st4 = es.enter_context(tc.tile_pool(name="st4", bufs=3))
        st5 = es.enter_context(tc.tile_pool(name="st5", bufs=3))
        ynp = es.enter_context(tc.tile_pool(name="ynp", bufs=3))
        pzp = es.enter_context(tc.tile_pool(name="pzp", bufs=3))
        trp = es.enter_context(tc.tile_pool(name="trp", bufs=6))
        op_ = es.enter_context(tc.tile_pool(name="outp", bufs=4))
        if True:
            # ---- load the input blob ----
            BL = cp.tile([128, NB], f16)
            nc.sync.dma_start(BL[:], dBLOB[:])
            EPS = cp.tile([128, 1], f32)
            nc.gpsimd.memset(EPS[:], LN_EPS)
            if need_lnsb or need_cb:
                CST = cp.tile([128, 4], f32)
                nc.sync.dma_start(CST[:], dCST[:])

            for i in range(NCH):
                t0 = TC * i
                # ---- G matmuls + tanh: 7 m-planes, each (Ge|Go) (128,1024) ----
                hts = []
                for m in range(K):
                    g = gp.tile([128, 1024], f32)
                    for ci, xoff in ((0, O_XE), (512, O_XO)):
                        for h in (0, 1):
                            p0 = 64 * h
                            nc.tensor.matmul(
                                g[p0:p0 + 64, ci:ci + TC],
                                lhsT=BL[p0:p0 + 64, O_WT + 64 * m:O_WT + 64 * m + 64],
                                rhs=BL[p0:p0 + 64, xoff + t0 + 6:xoff + t0 + 6 + TC],
                                start=True, stop=True,
                            )
                    ht = hp.tile([128, 1024], f16)
                    nc.scalar.activation(ht[:], g[:], AF.Tanh)
                    hts.append(ht)

                # ---- gating products (14 planes) ----
                # shifted tensors eliminated: for all m the window operand is
                # x*[:, t0+m : t0+m+TC] (odd m read the +1-shifted column
                # range of the same folded tensor)
                prods = []
                for m in range(K):
                    for ci, xoff in ((0, O_XE), (512, O_XO)):
                        pr = pp.tile([128, TC], f16)
                        nc.vector.tensor_mul(
                            pr[:], BL[:, xoff + t0 + m:xoff + t0 + m + TC],
                            hts[m][:, ci:ci + TC])
                        prods.append(pr)

                # ---- accumulate 14 products + skip via identity matmuls ----
                y = yp.tile([128, TC], f32)
                for j, pr in enumerate(prods):
                    nc.tensor.matmul(y[:], lhsT=BL[:, O_ID:O_ID + 128], rhs=pr[:],
                                     start=(j == 0), stop=False)
                nc.tensor.matmul(y[:], lhsT=BL[:, O_ID:O_ID + 128],
                                 rhs=BL[:, O_XE + t0 + 6:O_XE + t0 + 6 + TC],
                                 start=False, stop=True)

                # ---- drain y, square ----
                ysb = ypool.tile([128, TC], f16)
                nc.scalar.copy(ysb[:], y[:])
                ysq = pp.tile([128, TC], f16)
                nc.vector.tensor_mul(ysq[:], ysb[:], ysb[:])

                # ---- LN stats: mean & mean-of-squares via ones-matmul ----
                st = sp.tile([128, 1024], f32)
                for h in (0, 1):
                    p0 = 64 * h
                    nc.tensor.matmul(st[p0:p0 + 64, 0:TC],
                                     lhsT=BL[p0:p0 + 64, O_ON:O_ON + 64],
                                     rhs=ysb[p0:p0 + 64, :], start=True, stop=True)
                    nc.tensor.matmul(st[p0:p0 + 64, 512:512 + TC],
                                     lhsT=BL[p0:p0 + 64, O_ON:O_ON + 64],
                                     rhs=ysq[p0:p0 + 64, :], start=True, stop=True)
                mu = st[:, 0:TC]
                m2 = st[:, 512:512 + TC]

                musq = st1.tile([128, TC], f32)
                nc.scalar.activation(musq[:], mu, AF.Square)
                var = st2.tile([128, TC], f32)
                nc.vector.tensor_sub(var[:], m2, musq[:])
                std = st3.tile([128, TC], f32)
                nc.scalar.activation(std[:], var[:], AF.Sqrt, bias=EPS[:, 0:1])
                rstd = st4.tile([128, TC], f32)
                scr = st5.tile([128, TC], f32)
                nc.vector.reciprocal_approx_accurate(rstd[:], std[:], scr[:])

                # ---- yn = (y - mu) * rstd  (* s + b) ----
                yc = st1.tile([128, TC], f32)
                nc.vector.tensor_sub(yc[:], ysb[:], mu)
                yn = ynp.tile([128, TC], f16)
                nc.vector.tensor_mul(yn[:], yc[:], rstd[:])
                if need_lnsb:
                    yn2 = ynp.tile([128, TC], f16)
                    nc.vector.tensor_scalar(yn2[:], yn[:], CST[:, 0:1], CST[:, 1:2],
                                            op0=OP.mult, op1=OP.add)
                    yn = yn2

                # ---- 1x1 conv ----
                z = zp.tile([128, TC], f32)
                for h in (0, 1):
                    p0 = 64 * h
                    nc.tensor.matmul(z[p0:p0 + 64, :],
                                     lhsT=BL[p0:p0 + 64, O_CK:O_CK + 64],
                                     rhs=yn[p0:p0 + 64, :], start=True, stop=True)
                if need_cb:
                    z2 = st2.tile([128, TC], f32)
                    nc.vector.tensor_scalar(z2[:], z[:], CST[:, 2:3], None, op0=OP.add)
                    zsrc = z2
                else:
                    zsrc = z
                # prelu: max(z, slope*z)
                pz = pzp.tile([128, TC], f16)
                nc.scalar.activation(pz[:], zsrc[:], AF.Prelu,
                                     alpha=float(prelu_slope))

                # ---- transpose yn, pz to t-layout; add; store ----
                for h in (0, 1):
                    p0 = 64 * h
                    tb = HALF * h + t0
                    ynT = trp.tile([128, 4, 64], f16)
                    nc.sync.dma_start_transpose(ynT[:], yn[p0:p0 + 64, :])
                    pzT = trp.tile([128, 4, 64], f16)
                    nc.sync.dma_start_transpose(pzT[:], pz[p0:p0 + 64, :])
                    of = op_.tile([128, 4, 64], f16)
                    nc.vector.tensor_add(of[:], ynT[:], pzT[:])
                    if tb + TC <= T:
                        dst = dOUT[tb:tb + TC, :].rearrange(
                            "(j p) c -> p j c", p=128)
                        nc.sync.dma_start(dst, of[:])
                    else:
                        nfull = (T - tb) // 128
                        rem = (T - tb) - nfull * 128
                        if nfull > 0:
                            dst = dOUT[tb:tb + nfull * 128, :].rearrange(
                                "(j p) c -> p j c", p=128)
                            nc.sync.dma_start(dst, of[:, 0:nfull, :])
                        if rem > 0:
                            dst = dOUT[tb + nfull * 128:T, :]
                            nc.sync.dma_start(dst, of[0:rem, nfull, :])

    nc.compile()
    return nc


def _make_runner(nc, n_cores=8):
    """Cached jit of the bass program: one sharded executable reused across
    calls, donated output buffers generated on-device (no H2D of zeros),
    outputs fetched per-shard concurrently."""
    import jax
    import jax.numpy as jnp
    from jax.sharding import Mesh, PartitionSpec, NamedSharding
    from jax.experimental.shard_map import shard_map
    import concourse.mybir as mybir
    from concourse.bass2jax import (
        install_neuronx_cc_hook, _bass_exec_p, partition_id_tensor)

    install_neuronx_cc_hook()
    assert nc.dbg_addr is None

    partition_name = nc.partition_id_tensor.name if nc.partition_id_tensor else None
    in_names, out_names, out_avals = [], [], []
    for alloc in nc.m.functions[0].allocations:
        if not isinstance(alloc, mybir.MemoryLocationSet):
            continue
        name = alloc.memorylocations[0].name
        if alloc.kind == "ExternalInput":
            if name != partition_name:
                in_names.append(name)
        elif alloc.kind == "ExternalOutput":
            out_avals.append(jax.core.ShapedArray(
                tuple(alloc.tensor_shape), mybir.dt.np(alloc.dtype)))
            out_names.append(name)
    n_params = len(in_names)
    n_outs = len(out_avals)
    in_names_all = list(in_names) + list(out_names)
    if partition_name is not None:
        in_names_all.append(partition_name)

    def _body(*args):
        operands = list(args)
        if partition_name is not None:
            operands.append(partition_id_tensor())
        return tuple(_bass_exec_p.bind(
            *operands,
            out_avals=tuple(out_avals),
            in_names=tuple(in_names_all),
            out_names=tuple(out_names),
            lowering_input_output_aliases=(),
            sim_require_finite=True,
            sim_require_nnan=True,
            nc=nc,
        ))

    devices = jax.devices()[:n_cores]
    mesh = Mesh(np.asarray(devices), ("core",))
    in_specs = (PartitionSpec("core"),) * (n_params + n_outs)
    out_specs = (PartitionSpec("core"),) * n_outs
    donate = tuple(range(n_params, n_params + n_outs))
    sharded = jax.jit(
        shard_map(_body, mesh=mesh, in_specs=in_specs, out_specs=out_specs,
                  check_rep=False),
        donate_argnums=donate, keep_unused=True)

    zsh = tuple(NamedSharding(mesh, PartitionSpec("core")) for _ in out_avals)
    mkzeros = jax.jit(
        lambda: tuple(jnp.zeros((n_cores * a.shape[0], *a.shape[1:]), a.dtype)
                      for a in out_avals),
        out_shardings=zsh)

    def run(concat_inputs):
        """concat_inputs: list of np arrays, each (n_cores*per_core_rows, ...)
        in in_names order. Returns per-output list of per-core np arrays."""
        outs = sharded(*concat_inputs, *mkzeros())
        shard_lists = []
        for o in outs:
            shards = sorted(o.addressable_shards,
                            key=lambda s: s.index[0].start or 0)
            for s in shards:
                s.data.copy_to_host_async()
            shard_lists.append(shards)
        return [[np.asarray(s.data) for s in shards] for shards in shard_lists]

    return run, list(in_names)


def _prep_blob(x, weights, conv_kernel):
    """Host-side prep: one (8*128, NB) f16 blob, per-core rows stacked."""
    xf = np.asarray(x, dtype=np.float32)
    blob = np.zeros((B * 128, NB), np.float16)
    # shared consts, built once then broadcast to every core's rows
    wt = np.zeros((128, 64 * K), np.float16)
    for m in range(K):
        wmT = np.asarray(weights[:, :, m]).T.astype(np.float16)  # (c_in, d)
        wt[0:64, 64 * m:64 * m + 64] = wmT
        wt[64:128, 64 * m:64 * m + 64] = wmT
    ident = np.eye(128, dtype=np.float16)
    ones = np.full((128, 64), 1.0 / 64, np.float16)
    ck = np.zeros((128, 64), np.float16)
    ckc = np.asarray(conv_kernel).astype(np.float16)  # (c, o), lhsT layout
    ck[0:64] = ckc
    ck[64:128] = ckc
    consts = np.concatenate([wt, ident, ones, ck], axis=1)  # (128, NB-O_WT)

    for bi in range(B):
        r = bi * 128
        xb = xf[bi]                                          # (L, C)
        xeT = np.ascontiguousarray(xb[0::2].T).astype(np.float16)  # (64, 8192)
        xoT = np.ascontiguousarray(xb[1::2].T).astype(np.float16)
        blob[r:r + 64, O_XE:O_XE + WX] = xeT[:, 0:WX]
        blob[r + 64:r + 128, O_XE:O_XE + 8192 - HALF] = xeT[:, HALF:8192]
        blob[r:r + 64, O_XO:O_XO + WX] = xoT[:, 0:WX]
        blob[r + 64:r + 128, O_XO:O_XO + 8192 - HALF] = xoT[:, HALF:8192]
        blob[r:r + 128, O_WT:NB] = consts
    return blob


def _prep_csts(ln_scale, ln_bias, conv_bias):
    cst = np.zeros((128, 4), np.float32)
    s = np.asarray(ln_scale, np.float32)
    b = np.asarray(ln_bias, np.float32)
    cb = np.asarray(conv_bias, np.float32)
    cst[0:64, 0] = s
    cst[64:128, 0] = s
    cst[0:64, 1] = b
    cst[64:128, 1] = b
    cst[0:64, 2] = cb
    cst[64:128, 2] = cb
    cst[:, 3] = LN_EPS
    return np.concatenate([cst] * B, axis=0)  # (8*128, 4)


def _get(key):
    if key not in _CACHE:
        nc = _build(*key)
        run, in_names = _make_runner(nc)
        _CACHE[key] = (nc, run, in_names)
    return _CACHE[key]


def kernel(x, weights, ln_scale, ln_bias, conv_kernel, conv_bias, prelu_slope):
    slope = float(np.asarray(prelu_slope))
    need_lnsb = not (np.allclose(np.asarray(ln_scale), 1.0)
                     and np.allclose(np.asarray(ln_bias), 0.0))
    need_cb = not np.allclose(np.asarray(conv_bias), 0.0)

    nc, run, in_names = _get((slope, need_lnsb, need_cb))
    blob = _prep_blob(x, weights, conv_kernel)
    ins = {"blob": blob}
    if need_lnsb or need_cb:
        ins["csts"] = _prep_csts(ln_scale, ln_bias, conv_bias)
    outs = run([ins[n] for n in in_names])
    return np.stack(outs[0], axis=0).astype(np.float32)
